# revision 12
# baseline (speedup 1.0000x reference)
"""ChannelCovarianceBlock Trainium2 kernel (fp8 DoubleRow, all-resident).

Computes, for queries x1 (B, C, h, w) and support sets x2 (nw, Bs, C, h, w):
  cov_n = Cov(x2[n].reshape(Bs*C, hw))            (hw, hw) per class
  d     = normalize-and-center rows of x1.reshape(B*C, hw)
  sim[b, n, c] = d[bc] @ cov_n @ d[bc]^T          -> (B, nw*C)

Sharding: data-parallel over B across 8 NeuronCores (32 queries each);
each core computes all 10 class covariances from the full x2 (redundant
but collective-free) using the Gram identity cov = (X^T X - s s^T/N)/(N-1).

Design (v1 fp32r + D^T DRAM restream, 1729us -> this kernel, 953us):
  * Everything SBUF-resident: cov for all 10 classes (fp8, 55KB/part),
    D^T (fp8, 28KB/part), D (bf16, 50KB/part). Zero DRAM traffic after
    the initial input loads (v1 restreamed D^T 10x = 128MB/core).
  * fp8e4 DoubleRow matmuls (2 MACs/cell/cycle, K=256 per pass) for the
    Gram stage and the dominant S = D @ cov stage.
  * fp8 precision rescue: raw fp8 fails the 2e-2 gate (measured 2.1e-2)
    because cov ~ I and the diagonal ~1.0 quantizes at 6%. Fix: subtract
    SHIFT*I inside the PSUM accumulation (one bf16 identity matmul per
    cov tile), so fp8 stores cov' = cov - 1.0008*I with entries ~ +-0.05,
    then add back 1.0008*||d||^2 (computed exactly in f32) at the end.
    Power-of-2 pre-scales (x2*16, d*16, cov*64) dodge fp8 subnormals.
    Measured rel err 8.0e-3 (gate 2e-2).
  * Attempted and REVERTED (both measured slower on HW):
    - interleaving 4 psum groups across 2 classes to share LDWEIGHTS
      (953 -> 1492us: breaks the LDW/MMUL overlap; walrus emits an LDW
      per MMUL and the 4-way bank cycling serializes them), and
    - moving the stage-2 reduce+add and stage-0 centering to ACT
      (953 -> 1082us: ACT Identity table switching eats the gain).
"""

import os
import sys

for _p in ("/opt/trn_rl_repo", "/root/.axon_site/_ro/trn_rl_repo"):
    if os.path.isdir(_p) and _p not in sys.path:
        sys.path.append(_p)

import numpy as np

# Problem constants (hardcoded per spec).
B, C, H, W = 256, 128, 28, 28
NW, BS = 10, 10
HW = H * W            # 784
N_CORES = 8
BSH = B // N_CORES    # 32 queries per core
NI = BSH * C          # 4096 rows per core
NR = BS * C           # 1280 support rows per class

# K-tiles over the hw contraction dim (partition dim <= 128).
KT = [(k * 128, min(128, HW - k * 128)) for k in range((HW + 127) // 128)]
NKT = len(KT)         # 7 (6 full 128-tiles + one 16-row remainder)
NDR = 3               # DoubleRow pairs covering k-tiles 0..5
QT = [(0, 392), (392, 392)]
MT = NI // 128        # 32 i-tiles per core
RTN = NR // 128       # 10 support row-tiles per class

# fp8 scaling scheme (all powers of two; see module docstring).
XSC = 16.0                            # host-side x2 scale
DSC = 16.0                            # on-device d scale
CSC = 64.0                            # cov fp8 scale
SHIFT_PSUM = 327680.0                 # 5*2^16, bf16-exact; ~= XSC^2*(NR-1)
ADD_BACK = SHIFT_PSUM / (XSC * XSC * (NR - 1))   # 1.000782...
COV_MUL = CSC / (XSC * XSC * (NR - 1))           # PSUM -> cov8 scale
STT_MUL = 1.0 / (DSC * DSC * CSC)                # PSUM*d_res -> sim units
SSD_MUL = ADD_BACK / (DSC * DSC)                 # ACT-square accum -> add-back

_STATE = {}


def _build_program(mm_dtype_name=None, stages=None, repeat=None,
                   nw_count=NW, mt_count=MT):
    if stages is None:
        stages = os.environ.get("CCB_STAGES", "full")
    if repeat is None:
        repeat = int(os.environ.get("CCB_REPEAT", "1"))
    import concourse.bass as bass
    import concourse.bacc as bacc
    import concourse.tile as tile
    from concourse import mybir
    from concourse.masks import make_identity
    from contextlib import ExitStack

    f32 = mybir.dt.float32
    bf16 = mybir.dt.bfloat16
    f8 = mybir.dt.float8e4
    DR = mybir.MatmulPerfMode.DoubleRow

    nc = bacc.Bacc()
    x1s = nc.declare_dram_parameter("x1s", [NI, HW], f32, isOutput=False)
    x2d = nc.declare_dram_parameter("x2", [NW, NR, HW], f8, isOutput=False)
    out = nc.declare_dram_parameter("out", [NI, NW], f32, isOutput=True)

    AF = mybir.ActivationFunctionType
    OP = mybir.AluOpType

    with tile.TileContext(nc) as tc:
        with ExitStack() as ctx:
            persist = ctx.enter_context(tc.tile_pool(name="persist", bufs=1))
            ident = persist.tile([128, 128], f32, tag="ident")
            make_identity(nc, ident)
            ident_bf = persist.tile([128, 128], bf16, tag="ident_bf")
            nc.vector.tensor_copy(out=ident_bf, in_=ident)
            # pair-dim stride must be 16B-aligned for DoubleRow ldweights,
            # so pad the ones tile to [128, 2, 16] and slice [:, :, :1].
            ones_f = persist.tile([128, 2, 16], f32, tag="ones_f")
            nc.vector.memset(ones_f, 1.0)
            ones8 = persist.tile([128, 2, 16], f8, tag="ones8")
            nc.vector.tensor_copy(out=ones8, in_=ones_f)
            # sid[k, mc, q] = -SHIFT_PSUM * delta(q == mc*128 + k): the
            # rank-128 identity block used to shift cov's diagonal inside
            # the PSUM accumulation.
            sid = persist.tile([128, NKT, HW], bf16, tag="sid")
            nc.vector.memset(sid, 0.0)
            for mc, (mo, ml) in enumerate(KT):
                nc.scalar.mul(out=sid[:, mc, mo:mo + ml],
                              in_=ident[:, :ml], mul=-SHIFT_PSUM)
            # D resident both ways: rows for the final elementwise reduce,
            # transposed fp8 (scaled 16x) as stage-2 matmul weights.
            d_res = persist.tile([128, MT, HW], bf16, tag="d_res")
            dtT = persist.tile([128, MT, NKT, 128], f8, tag="dtT")
            # cov' fp8 (scaled 64x) for all classes, k-tile-major rows.
            cov8 = persist.tile([128, NW, NKT, HW], f8, tag="cov8")
            ssd = persist.tile([128, MT], f32, tag="ssd")
            out_acc = persist.tile([128, MT, NW], f32, tag="out_acc")
            if nw_count < NW or mt_count < MT or stages != "full":
                nc.vector.memset(out_acc, 0.0)  # reduced/probe builds

            xw_pool = ctx.enter_context(tc.tile_pool(name="xw", bufs=2))
            stats = ctx.enter_context(tc.tile_pool(name="stats", bufs=6))
            scr_pool = ctx.enter_context(tc.tile_pool(name="scr", bufs=2))
            s8_pool = ctx.enter_context(tc.tile_pool(name="scr8", bufs=2))
            xs_pool = ctx.enter_context(tc.tile_pool(name="xsup", bufs=2))
            row_pool = ctx.enter_context(tc.tile_pool(name="rows", bufs=2))

            # Optional on-device repeat loop (timing amplification only).
            if repeat > 1:
                ctx.enter_context(tc.For_i(0, repeat, 1))

            # ---- Stage 0: query preprocessing, d (bf16) + 16*d^T (fp8) ----
            with tc.tile_pool(name="psum_t", bufs=2, space="PSUM") as psum_t:
                for m in range(mt_count):
                    rows = slice(m * 128, (m + 1) * 128)
                    xw = xw_pool.tile([128, HW], f32, tag="xw")
                    nc.sync.dma_start(out=xw, in_=x1s[rows, :])
                    sq = scr_pool.tile([128, HW], f32, tag="scr")
                    sumsq = stats.tile([128, 1], f32, tag="sumsq")
                    # ACT: sq = x^2 (discarded), sumsq = row-sum(x^2)
                    nc.scalar.activation(
                        out=sq, in_=xw, func=AF.Square, accum_out=sumsq
                    )
                    s1 = stats.tile([128, 1], f32, tag="s1")
                    nc.vector.tensor_reduce(
                        out=s1, in_=xw, axis=mybir.AxisListType.X, op=OP.add
                    )
                    # nrm16 = ||x|| / 16, rn = 16 / ||x||
                    nrm16 = stats.tile([128, 1], f32, tag="nrm")
                    nc.scalar.activation(out=nrm16, in_=sumsq, func=AF.Sqrt,
                                         scale=1.0 / (DSC * DSC))
                    rn = stats.tile([128, 1], f32, tag="rn")
                    nc.vector.reciprocal(out=rn, in_=nrm16)
                    ms = stats.tile([128, 1], f32, tag="ms")
                    nc.scalar.mul(out=ms, in_=s1, mul=1.0 / HW)
                    # d_res[:, m] = (x - mean) * (16/||x||) = 16 * d, written
                    # directly as bf16 (bf16 PE transposes run 1 cyc/row vs
                    # fp32's 2, and the separate f32 staging copy goes away)
                    nc.vector.tensor_scalar(
                        out=d_res[:, m, :], in0=xw, scalar1=ms, scalar2=rn,
                        op0=OP.subtract, op1=OP.mult,
                    )
                    # ssd[:, m] = 1.0008 * ||d||^2 (from 256*||d||^2 accum)
                    sq2 = scr_pool.tile([128, HW], f32, tag="scr")
                    ssd2 = stats.tile([128, 1], f32, tag="ssd2")
                    nc.scalar.activation(
                        out=sq2, in_=d_res[:, m, :], func=AF.Square,
                        accum_out=ssd2
                    )
                    nc.scalar.mul(out=ssd[:, m:m + 1], in_=ssd2, mul=SSD_MUL)
                    for kt, (ko, kl) in enumerate(KT):
                        pt = psum_t.tile([128, 128], bf16, tag="pt")
                        nc.tensor.transpose(
                            out=pt[:kl, :128],
                            in_=d_res[:, m, ko:ko + kl],
                            identity=ident_bf,
                        )
                        nc.vector.tensor_copy(out=dtT[:kl, m, kt, :],
                                              in_=pt[:kl, :128])

            # ---- Stage 1 (per class): cov' = cov - 1.0008*I in fp8 ----
            psum_m = ctx.enter_context(
                tc.tile_pool(name="psum_m", bufs=1, space="PSUM"))
            psum_s = ctx.enter_context(
                tc.tile_pool(name="psum_s", bufs=6, space="PSUM"))
            for n in range(nw_count if stages != "0" else 0):
                xs = xs_pool.tile([128, RTN, HW], f8, tag="xs")
                for rt in range(RTN):
                    nc.sync.dma_start(
                        out=xs[:, rt, :],
                        in_=x2d[n, rt * 128:(rt + 1) * 128, :])
                # column sums of 16*x via fp8 DoubleRow ones-matmul
                pm = psum_m.tile([1, 2, 512], f32, tag="pm")
                for j in range(RTN // 2):
                    for qi, (qo, ql) in enumerate(QT):
                        nc.tensor.matmul(
                            pm[:1, qi, :ql], lhsT=ones8[:, :, :1],
                            rhs=xs[:, 2 * j:2 * j + 2, qo:qo + ql],
                            start=(j == 0), stop=(j == RTN // 2 - 1),
                            perf_mode=DR,
                        )
                srow = row_pool.tile([1, HW], bf16, tag="srow")
                ssrow = row_pool.tile([1, HW], bf16, tag="ssrow")
                for qi, (qo, ql) in enumerate(QT):
                    nc.scalar.mul(out=srow[:, qo:qo + ql],
                                  in_=pm[:1, qi, :ql], mul=1.0)
                    nc.scalar.mul(out=ssrow[:, qo:qo + ql],
                                  in_=pm[:1, qi, :ql], mul=-1.0 / NR)
                for mc, (mo, ml) in enumerate(KT):
                    ps2 = [psum_s.tile([128, 392], f32, name="ps", tag="ps")
                           for _ in QT]
                    for j in range(RTN // 2):
                        for qi, (qo, ql) in enumerate(QT):
                            nc.tensor.matmul(
                                ps2[qi][:ml, :ql],
                                lhsT=xs[:, 2 * j:2 * j + 2, mo:mo + ml],
                                rhs=xs[:, 2 * j:2 * j + 2, qo:qo + ql],
                                start=(j == 0), stop=False, perf_mode=DR,
                            )
                    for qi, (qo, ql) in enumerate(QT):
                        # diagonal shift (only where block mc overlaps qh)
                        if mo < qo + ql and mo + ml > qo:
                            nc.tensor.matmul(
                                ps2[qi][:ml, :ql], lhsT=ident_bf[:, :ml],
                                rhs=sid[:, mc, qo:qo + ql],
                                start=False, stop=False,
                            )
                        # rank-1 mean correction: -= s s^T / NR
                        nc.tensor.matmul(
                            ps2[qi][:ml, :ql], lhsT=ssrow[:1, mo:mo + ml],
                            rhs=srow[:1, qo:qo + ql],
                            start=False, stop=True,
                        )
                        nc.scalar.mul(out=cov8[:ml, n, mc, qo:qo + ql],
                                      in_=ps2[qi][:ml, :ql], mul=COV_MUL)

            # ---- Stage 2: sim = rowsum((16d @ cov') * 16d)/2^14 + ssd ----
            klast = KT[-1][1]
            for m in range(mt_count if stages not in ("0", "01") else 0):
                for n in range(nw_count):
                    ps2 = [psum_s.tile([128, 392], f32, name="ps", tag="ps")
                           for _ in QT]
                    for j in range(NDR):
                        for qi, (qo, ql) in enumerate(QT):
                            nc.tensor.matmul(
                                ps2[qi][:, :ql],
                                lhsT=dtT[:, m, 2 * j:2 * j + 2, :],
                                rhs=cov8[:, n, 2 * j:2 * j + 2, qo:qo + ql],
                                start=(j == 0), stop=False, perf_mode=DR,
                            )
                    for qi, (qo, ql) in enumerate(QT):
                        nc.tensor.matmul(
                            ps2[qi][:, :ql], lhsT=dtT[:klast, m, NKT - 1, :],
                            rhs=cov8[:klast, n, NKT - 1, qo:qo + ql],
                            start=False, stop=True,
                        )
                    pp = stats.tile([128, 2], f32, tag="pp")
                    for qi, (qo, ql) in enumerate(QT):
                        scr8 = s8_pool.tile([128, 392], f8, tag="s8")
                        nc.vector.scalar_tensor_tensor(
                            out=scr8[:, :ql], in0=ps2[qi][:, :ql],
                            scalar=STT_MUL, in1=d_res[:, m, qo:qo + ql],
                            op0=OP.mult, op1=OP.mult,
                            accum_out=pp[:, qi:qi + 1],
                        )
                    red = stats.tile([128, 1], f32, tag="red")
                    nc.vector.tensor_reduce(
                        out=red, in_=pp, axis=mybir.AxisListType.X, op=OP.add
                    )
                    nc.vector.tensor_scalar(
                        out=out_acc[:, m, n:n + 1], in0=red,
                        scalar1=ssd[:, m:m + 1], scalar2=None, op0=OP.add,
                    )

            for m in range(mt_count):
                nc.sync.dma_start(
                    out=out[m * 128:(m + 1) * 128, :], in_=out_acc[:, m, :]
                )

    # Bacc defers register allocation to compile(); run_bass_via_pjrt
    # serializes the module as-is, so finalize here.
    nc.finalize()
    return nc


def get_program():
    key = "nc"
    if key not in _STATE:
        _STATE[key] = _build_program()
    return _STATE[key]


def make_in_maps(x1, x2):
    import ml_dtypes
    x1f = np.ascontiguousarray(
        np.asarray(x1, dtype=np.float32).reshape(B * C, HW)
    )
    x2q = np.ascontiguousarray(
        (np.asarray(x2, dtype=np.float32).reshape(NW, NR, HW) * XSC)
        .astype(ml_dtypes.float8_e4m3)
    )
    return [
        {"x1s": x1f[c * NI:(c + 1) * NI], "x2": x2q}
        for c in range(N_CORES)
    ]


def assemble_output(core_outs):
    # per-core (NI, NW) -> (BSH, NW*C); concat over cores -> (B, NW*C)
    parts = [
        o.reshape(BSH, C, NW).transpose(0, 2, 1).reshape(BSH, NW * C)
        for o in core_outs
    ]
    return np.ascontiguousarray(np.concatenate(parts, axis=0), dtype=np.float32)


def kernel(x1, x2):
    from concourse.bass_utils import run_bass_kernel_spmd

    nc = get_program()
    in_maps = make_in_maps(x1, x2)
    res = run_bass_kernel_spmd(nc, in_maps, list(range(N_CORES)))
    return assemble_output([res.results[i]["out"] for i in range(N_CORES)])
